# revision 9
# baseline (speedup 1.0000x reference)
"""LSTM cell (batch 8192, input 512, hidden 512) on 8 Trainium2 NeuronCores.

Data-parallel over the batch dim: each core handles 1024 rows. Weights are
replicated. The host pre-transposes both matmul operands so the contraction
dim (fan_in = 1024) lands on SBUF partitions:

  gate.T[n, b] = sum_k W.T[k, n] * combined.T[k, b]     (matmul: lhsT.T @ rhs)

so the kernel computes everything in [hidden, batch] layout; gate biases
become per-partition vectors (free on the ACT activation op), and the host
transposes the outputs back after the gather.

Matmul operands are cast to bf16 on the host (f32 matmul on PE is 4x slower
per the cost model); accumulation is f32 in PSUM and the whole elementwise
tail (c_next = f*c_prev + i*c_tilde, h_next = o*tanh(c_next)) stays f32.
"""

import numpy as np

import concourse.bacc as bacc
import concourse.bass as bass
import concourse.mybir as mybir
from concourse import tile
from concourse.bass_utils import run_bass_kernel_spmd

N_CORES = 8
BATCH = 8192
B = BATCH // N_CORES  # 1024 batch rows per core
K = 1024              # fan_in = input_dim + hidden_dim
H = 512               # hidden dim
NG = 4                # gates: i, f, c, o
KT = K // 128         # 8 contraction tiles
HT = H // 128         # 4 hidden chunks per gate
BT = B // 512         # 2 batch halves (PSUM free-dim limit is 512 f32)

MM_DT = mybir.dt.bfloat16
F32 = mybir.dt.float32

_SIG = mybir.ActivationFunctionType.Sigmoid
_TANH = mybir.ActivationFunctionType.Tanh
# gate order within the concatenated weight: i, f, c, o
_GATE_FN = [_SIG, _SIG, _TANH, _SIG]


def _build():
    nc = bacc.Bacc(
        "TRN2",
        target_bir_lowering=False,
        debug=False,
        num_devices=N_CORES,
    )

    xhT = nc.dram_tensor("xhT", [K, B], MM_DT, kind="ExternalInput")
    wT = nc.dram_tensor("wT", [K, NG * H], MM_DT, kind="ExternalInput")
    bias2d = nc.dram_tensor("bias2d", [128, NG * HT], F32, kind="ExternalInput")
    c_prevT = nc.dram_tensor("c_prevT", [H, B], F32, kind="ExternalInput")
    h_nextT = nc.dram_tensor("h_nextT", [H, B], F32, kind="ExternalOutput")
    c_nextT = nc.dram_tensor("c_nextT", [H, B], F32, kind="ExternalOutput")

    with tile.TileContext(nc) as tc:
        with (
            tc.tile_pool(name="wts", bufs=1) as wpool,
            tc.tile_pool(name="acts", bufs=1) as apool,
            tc.tile_pool(name="cprev", bufs=1) as cpool,
            tc.tile_pool(name="gates", bufs=3) as gpool,
            tc.tile_pool(name="ew", bufs=3) as epool,
            tc.tile_pool(name="psum", bufs=1, space="PSUM") as pspool,
        ):
            bias_t = wpool.tile([128, NG * HT], F32, tag="bias")
            nc.sync.dma_start(bias_t[:], bias2d[:])

            # Stream inputs k-major so the first accumulation groups can
            # start before the full weight set has landed.
            xh_tiles = []
            wt_tiles = []  # [k][g] -> [128, H] tile
            for k in range(KT):
                xt = apool.tile([128, B], MM_DT, tag=f"xh{k}")
                nc.sync.dma_start(xt[:], xhT[k * 128:(k + 1) * 128, :])
                xh_tiles.append(xt)
                per_g = []
                for g in range(NG):
                    wt = wpool.tile([128, H], MM_DT, tag=f"w{k}_{g}")
                    nc.sync.dma_start(
                        wt[:], wT[k * 128:(k + 1) * 128, g * H:(g + 1) * H]
                    )
                    per_g.append(wt)
                wt_tiles.append(per_g)

            cp_tiles = []
            for h in range(HT):
                ct = cpool.tile([128, B], F32, tag=f"cp{h}")
                nc.sync.dma_start(ct[:], c_prevT[h * 128:(h + 1) * 128, :])
                cp_tiles.append(ct)

            for h in range(HT):
                for b2 in range(BT):
                    bs = slice(b2 * 512, (b2 + 1) * 512)
                    # 4 gate accumulation groups per (h, b2); b2 parity
                    # alternates between two 4-bank PSUM sets so the ACT
                    # drain of one set overlaps the next set's matmuls.
                    psum = [
                        pspool.tile(
                            [128, 512], F32,
                            tag=f"ps{g}_{b2 % 2}", name=f"ps{g}_{h}_{b2}",
                        )
                        for g in range(NG)
                    ]
                    for k in range(KT):
                        for g in range(NG):
                            nc.tensor.matmul(
                                psum[g][:],
                                wt_tiles[k][g][:, h * 128:(h + 1) * 128],
                                xh_tiles[k][:, bs],
                                start=(k == 0),
                                stop=(k == KT - 1),
                            )

                    gt = []
                    for g in range(NG):
                        t = gpool.tile([128, 512], F32, tag=f"g{g}", name=f"g{g}_{h}_{b2}")
                        nc.scalar.activation(
                            t[:], psum[g][:], _GATE_FN[g],
                            bias=bias_t[:, g * HT + h:g * HT + h + 1],
                        )
                        gt.append(t)
                    gi, gf, gc, go = gt

                    t1 = epool.tile([128, 512], F32, tag="t1")
                    nc.vector.tensor_mul(t1[:], gi[:], gc[:])           # i * c~
                    t2 = epool.tile([128, 512], F32, tag="t2")
                    nc.vector.tensor_mul(t2[:], gf[:], cp_tiles[h][:, bs])  # f * c_prev
                    cn = epool.tile([128, 512], F32, tag="cn")
                    nc.vector.tensor_add(cn[:], t1[:], t2[:])
                    nc.gpsimd.dma_start(c_nextT[h * 128:(h + 1) * 128, bs], cn[:])

                    th = epool.tile([128, 512], F32, tag="th")
                    nc.scalar.activation(th[:], cn[:], _TANH)
                    hn = epool.tile([128, 512], F32, tag="hn")
                    nc.vector.tensor_mul(hn[:], go[:], th[:])
                    nc.gpsimd.dma_start(h_nextT[h * 128:(h + 1) * 128, bs], hn[:])

    nc.compile()
    return nc


_NC_CACHE = None
_LAST_IN_MAPS = None


def kernel(x, h_prev, c_prev, W_i, b_i, W_f, b_f, W_c, b_c, W_o, b_o):
    global _NC_CACHE, _LAST_IN_MAPS
    if _NC_CACHE is None:
        _NC_CACHE = _build()
    nc = _NC_CACHE

    np_bf16 = mybir.dt.np(MM_DT)

    combT = np.concatenate([x, h_prev], axis=1).T          # (K, BATCH) f32
    combT = combT.astype(np_bf16)
    wT = np.ascontiguousarray(
        np.concatenate([W_i, W_f, W_c, W_o], axis=0).T     # (K, 4H)
    ).astype(np_bf16)
    bias2d = np.ascontiguousarray(
        np.concatenate([b_i, b_f, b_c, b_o]).reshape(NG * HT, 128).T
    ).astype(np.float32)                                   # (128, 16)
    c_prevT = c_prev.T                                     # (H, BATCH)

    in_maps = []
    for j in range(N_CORES):
        cols = slice(j * B, (j + 1) * B)
        in_maps.append({
            "xhT": np.ascontiguousarray(combT[:, cols]),
            "wT": wT,
            "bias2d": bias2d,
            "c_prevT": np.ascontiguousarray(c_prevT[:, cols], dtype=np.float32),
        })

    _LAST_IN_MAPS = in_maps
    res = run_bass_kernel_spmd(nc, in_maps, core_ids=list(range(N_CORES)))

    h_next = np.concatenate([r["h_nextT"].T for r in res.results], axis=0)
    c_next = np.concatenate([r["c_nextT"].T for r in res.results], axis=0)
    return (h_next.astype(np.float32), c_next.astype(np.float32))
